# revision 4
# baseline (speedup 1.0000x reference)
"""Trainium2 Bass kernel for causal multi-head attention (b=2, n=2048, d=1024, h=16).

Sharding: 8 cores; core c handles batch (c // 4) and the 4 heads
[4*(c%4), 4*(c%4)+4).  Each core computes its heads' attention plus its
partial output projection y_part = O_heads @ Wo[:, cols].T ; the host sums
the 4 per-batch partials and adds bo (with the V-bias contribution folded
in host-side: softmax rows sum to 1, so bv contributes bv @ Wo.T).

All matmul operands are bf16 (fp32 PSUM accumulation); inputs are converted
host-side, halving the startup DMA vs f32r and enabling fast weight load.

On-device pipeline per core:
  xT (d-major, bf16) -> QTz, KT [hd, n] and V [n, hd] projections (bf16)
  ST tile [k,q] = KT-chunk.T x QTz_h       (K=128: both heads' KT rows with
                                            the other head's QT rows zeroed;
                                            1/8 scale folded into QT)
  PT = exp(ST) in bf16 (no max subtraction; scores are O(10), fp32-exp safe)
  causal masking: the diagonal 128-col triangle of each diagonal chunk is
  multiplied with a single [128,128] 0/1 bf16 mask block
  OT_aug [65, q] += V_aug-chunk.T x PT     (V_aug = [V | ones]; row 64 = l)
  per head: rl = 1/l read straight from PSUM, DRAM-bounce broadcast,
  OTn2 pair tile [128, n] = OT * bcast(1/l)  (heads 2p, 2p+1 stacked)
  y[tok, :] += OTn2-pair-chunk.T x Wo_pair (K=128, no zero rows)

Hardware rules honored (measured / verifier-enforced):
  - matmuls keep K=128: sustained K<128 matmuls run at the cold 1.2 GHz PE
    clock, doubling their cost.
  - 1/l is broadcast across partitions with a DRAM-bounce DMA, not the PE.
  - PSUM is not DMA-accessible: y goes PSUM -> copy (alternating DVE/ACT)
    -> SBUF -> DMA.
"""

import numpy as np

import concourse.bass as bass
import concourse.mybir as mybir
import concourse.tile as tile
from concourse import bacc
from concourse.bass_utils import run_bass_kernel_spmd

D = 1024          # d_model
N = 2048          # sequence length
B = 2             # batch
H_TOT = 16        # total heads
HD = 64           # head dim
HPC = 4           # heads per core
NCORES = 8
SCALE = HD ** -0.5

F32 = mybir.dt.float32
BF16 = mybir.dt.bfloat16

QTILE = 512       # q-tile width (free dim of score matmuls)
KCH = 128         # k-chunk (partition dim of score tiles)
NQT = N // QTILE  # 4
NKC = N // KCH    # 16
DCH = D // 128    # 8 d_model chunks
VROW = HD + 1     # V columns per head incl. ones column
NJUNK = 32        # PE warm-up matmuls (cover startup DMA @ ~6.8us)


def build_kernel():
    nc = bacc.Bacc("TRN2", target_bir_lowering=False, debug=False,
                   num_devices=NCORES)

    xT = nc.dram_tensor("xT", [D, N], BF16, kind="ExternalInput").ap()
    wq = nc.dram_tensor("wqT", [D, HPC * HD], BF16, kind="ExternalInput").ap()
    wk = nc.dram_tensor("wkT", [D, HPC * HD], BF16, kind="ExternalInput").ap()
    wv = nc.dram_tensor("wvT", [D, HPC * HD], BF16, kind="ExternalInput").ap()
    wo = nc.dram_tensor("woT", [HPC * HD, D], BF16, kind="ExternalInput").ap()
    bqz = nc.dram_tensor("bqz", [128, HPC], F32, kind="ExternalInput").ap()
    sclz = nc.dram_tensor("sclz", [128, HPC], F32, kind="ExternalInput").ap()
    bkd = nc.dram_tensor("bk", [HPC * HD], F32, kind="ExternalInput").ap()
    maskd = nc.dram_tensor("mask", [128, 128], BF16, kind="ExternalInput").ap()
    y = nc.dram_tensor("y", [N, D], F32, kind="ExternalOutput").ap()

    Exp = mybir.ActivationFunctionType.Exp
    Identity = mybir.ActivationFunctionType.Identity

    with tile.TileContext(nc) as tc:
        from contextlib import ExitStack
        with ExitStack() as ctx:
            singles = ctx.enter_context(tc.tile_pool(name="singles", bufs=1))
            pt_pool = ctx.enter_context(tc.tile_pool(name="pt", bufs=4))
            r_pool = ctx.enter_context(tc.tile_pool(name="rp", bufs=2))
            yout = ctx.enter_context(tc.tile_pool(name="yout", bufs=3))
            dram = ctx.enter_context(
                tc.tile_pool(name="dram", bufs=2, space="DRAM"))
            ps_mm = ctx.enter_context(
                tc.tile_pool(name="psmm", bufs=2, space="PSUM"))
            ps_st = ctx.enter_context(
                tc.tile_pool(name="psst", bufs=2, space="PSUM"))
            ps_ot = ctx.enter_context(
                tc.tile_pool(name="psot", bufs=2, space="PSUM"))

            # --- resident inputs (DMA order = priority: first MMs need
            # wq/wk + x block 0) -------------------------------------------
            wq_sb = singles.tile([128, DCH, HPC * HD], BF16)
            nc.sync.dma_start(wq_sb[:], wq.rearrange("(o p) m -> p o m", p=128))
            wk_sb = singles.tile([128, DCH, HPC * HD], BF16)
            nc.sync.dma_start(wk_sb[:], wk.rearrange("(o p) m -> p o m", p=128))
            bqz_sb = singles.tile([128, HPC], F32)
            nc.sync.dma_start(bqz_sb[:], bqz)
            sclz_sb = singles.tile([128, HPC], F32)
            nc.sync.dma_start(sclz_sb[:], sclz)
            bk_sb = singles.tile([128, 2], F32)
            nc.sync.dma_start(bk_sb[:], bkd.rearrange("(o p) -> p o", p=128))

            # x, sliced [d-chunk, 512-token block] so compute can start as
            # soon as the first block's 1 MB lands.
            xk = [[singles.tile([128, QTILE], BF16, name=f"xk{k}_{b}")
                   for b in range(NQT)] for k in range(DCH)]
            for k in range(DCH):
                nc.sync.dma_start(
                    xk[k][0][:], xT[k * 128:(k + 1) * 128, 0:QTILE])

            wv_sb = singles.tile([128, DCH, HPC * HD], BF16)
            nc.sync.dma_start(wv_sb[:], wv.rearrange("(o p) m -> p o m", p=128))
            for k in range(DCH):
                nc.sync.dma_start(
                    xk[k][1][:], xT[k * 128:(k + 1) * 128, QTILE:2 * QTILE])
            wo2 = []
            for p in range(2):
                t = singles.tile([128, D], BF16, name=f"wo{p}")
                nc.sync.dma_start(t[:], wo[p * 128:(p + 1) * 128, :])
                wo2.append(t)
            for b in range(2, NQT):
                for k in range(DCH):
                    nc.sync.dma_start(
                        xk[k][b][:],
                        xT[k * 128:(k + 1) * 128,
                           b * QTILE:(b + 1) * QTILE])
            mask_sb = singles.tile([128, 128], BF16)
            nc.sync.dma_start(mask_sb[:], maskd)

            # PE warm-up: the first few us are DMA-bound with the PE idle,
            # which leaves the PE clock throttled to 1.2 GHz when real work
            # starts.  Issue dependency-free junk matmuls so the activity
            # monitor unthrottles before the first projection matmul.
            junk = singles.tile([128, 512], BF16)
            nc.vector.memset(junk[:], 0.0)
            for i in range(NJUNK):
                wps = ps_ot.tile([128, 512], F32, tag="ot", name="wps")
                nc.tensor.matmul(wps[:], lhsT=junk[:, :128], rhs=junk[:],
                                 start=True, stop=True)

            # QTz[h][qi]: [128, 512] with head h's 64 rows live at partition
            # offset (h%2)*64 and the other 64 rows zero, so score matmuls
            # contract over the full 128 partitions.
            QTz = [[singles.tile([128, QTILE], BF16, name=f"qtz{h}_{i}")
                    for i in range(NQT)] for h in range(HPC)]
            KT_sb = [singles.tile([128, 2, QTILE], BF16, name=f"kt{i}")
                     for i in range(NQT)]
            V_sb = [singles.tile([128, 4, HPC, VROW], BF16, name=f"v{i}")
                    for i in range(NQT)]
            # OTn2[p]: heads 2p (rows 0..63) and 2p+1 (rows 64..127) stacked
            # so the output projection contracts over K=128 with no zeros.
            OTn2 = [singles.tile([128, N], BF16, name=f"otn{p}")
                    for p in range(2)]

            # --- stage A: projections --------------------------------------
            for blk in range(NQT):
                for m in range(2):
                    ps = ps_mm.tile([128, 512], F32, tag="mm")
                    for k in range(DCH):
                        nc.tensor.matmul(
                            ps[:],
                            lhsT=wq_sb[:, k, m * 128:(m + 1) * 128],
                            rhs=xk[k][blk][:],
                            start=(k == 0), stop=(k == DCH - 1))
                    for hh in range(2):
                        h = 2 * m + hh
                        # per-partition scale zeroes the other head's rows
                        nc.scalar.activation(
                            QTz[h][blk][:], ps[:], Identity,
                            bias=bqz_sb[:, h:h + 1],
                            scale=sclz_sb[:, h:h + 1])
                for m in range(2):
                    ps = ps_mm.tile([128, 512], F32, tag="mm")
                    for k in range(DCH):
                        nc.tensor.matmul(
                            ps[:],
                            lhsT=wk_sb[:, k, m * 128:(m + 1) * 128],
                            rhs=xk[k][blk][:],
                            start=(k == 0), stop=(k == DCH - 1))
                    nc.scalar.activation(
                        KT_sb[blk][:, m, :], ps[:], Identity,
                        bias=bk_sb[:, m:m + 1], scale=1.0)
                nc.vector.memset(V_sb[blk][:, :, :, HD], 1.0)
                for tt in range(4):
                    ps = ps_mm.tile([128, 512], F32, tag="mm")
                    for k in range(DCH):
                        nc.tensor.matmul(
                            ps[:, :HPC * HD],
                            lhsT=xk[k][blk][:, tt * 128:(tt + 1) * 128],
                            rhs=wv_sb[:, k, :],
                            start=(k == 0), stop=(k == DCH - 1))
                    nc.vector.tensor_copy(
                        V_sb[blk][:, tt, :, :HD], ps[:, :HPC * HD])

            # --- stages B+D: attention, then the block's output projection -
            for qi in range(NQT):
                q0 = qi * QTILE
                for h in range(HPC):
                    mi = h // 2
                    nprs = 2 * (qi + 1)        # pairs of 128-k-chunks
                    pso = ps_ot.tile([VROW, 512], F32, tag="ot", name="pso")

                    pair_ps = [None] * nprs
                    pair_pt = [None] * nprs

                    def consume(pi):
                        # last pair of each q-tile: only columns q >= r are
                        # unmasked (r = 256, 384); compute just those.
                        shrunk = (pi == nprs - 1)
                        pss, pt = pair_ps[pi], pair_pt[pi]
                        if shrunk:
                            for j in range(2):
                                r = (2 * pi + j) * KCH - q0
                                nc.scalar.activation(
                                    pt[:, j, r:], pss[:, j, r:], Exp)
                        else:
                            nc.scalar.activation(pt[:], pss[:], Exp)
                        for j in range(2):
                            ki = 2 * pi + j
                            r = ki * KCH - q0
                            if r >= 0:
                                # causal triangle: only the 128 cols at the
                                # diagonal need masking; the same [128,128]
                                # block works for every offset r.
                                nc.vector.tensor_mul(
                                    pt[:, j, r:r + KCH], pt[:, j, r:r + KCH],
                                    mask_sb[:])
                        for j in range(2):
                            ki = 2 * pi + j
                            s = max(ki * KCH - q0, 0)
                            nc.tensor.matmul(
                                pso[:, s:],
                                lhsT=V_sb[ki // 4][:, ki % 4, h, :],
                                rhs=pt[:, j, s:],
                                start=(ki == 0), stop=(ki == 4 * (qi + 1) - 1))

                    for pi in range(nprs):
                        pss = ps_st.tile([128, 2, 512], F32, tag="st",
                                         name="pss")
                        pair_ps[pi] = pss
                        pair_pt[pi] = pt_pool.tile([128, 2, 512], BF16,
                                                   tag="pt", name="pt")
                        for j in range(2):
                            ki = 2 * pi + j
                            s = (ki * KCH - q0) if pi == nprs - 1 else 0
                            nc.tensor.matmul(
                                pss[:, j, s:],
                                lhsT=KT_sb[ki // 4][:, mi,
                                                    (ki % 4) * 128:
                                                    (ki % 4) * 128 + 128],
                                rhs=QTz[h][qi][:, s:],
                                start=True, stop=True)
                        if pi > 0:
                            consume(pi - 1)
                    consume(nprs - 1)

                    # Normalize: 1/l read straight from the PSUM l-row,
                    # broadcast across 64 partitions via a DRAM bounce, then
                    # one fused PSUM-read multiply into the pair tile.
                    p = h // 2
                    po = (h % 2) * HD
                    lw = r_pool.tile([1, 512], F32, tag="lw", name="lw")
                    nc.vector.tensor_copy(lw[:], pso[HD:HD + 1, :])
                    rl = r_pool.tile([1, 512], F32, tag="rl", name="rl")
                    nc.vector.reciprocal_approx_fast(out=rl[:], in_=lw[:])
                    sc = dram.tile([1, 512], F32, tag="sc", name="sc")
                    nc.sync.dma_start(sc[:], rl[:])
                    rb = r_pool.tile([HD, 512], F32, tag="rb", name="rb")
                    row = sc[0, :]
                    bcast = bass.AP(tensor=row.tensor, offset=row.offset,
                                    ap=[[0, HD]] + list(row.ap))
                    nc.sync.dma_start(rb[:], bcast)
                    nc.vector.tensor_mul(
                        OTn2[p][po:po + HD, q0:q0 + QTILE], pso[:HD, :],
                        rb[:])

                # output projection for this q-block's 4 token chunks
                for tt in range(4):
                    t0 = q0 + tt * 128
                    for half in range(2):
                        # last block: attention is over, so its score-psum
                        # slots are free — borrow them to double the stage-D
                        # rotation while the final head's normalize lands.
                        if qi == NQT - 1 and (tt * 2 + half) % 2 == 1:
                            ps = ps_st.tile([128, 512], F32, tag="st",
                                            name="psy")
                        else:
                            ps = ps_mm.tile([128, 512], F32, tag="mm",
                                            name="psy")
                        for p in range(2):
                            nc.tensor.matmul(
                                ps[:],
                                lhsT=OTn2[p][:, t0:t0 + 128],
                                rhs=wo2[p][:, half * 512:half * 512 + 512],
                                start=(p == 0), stop=(p == 1))
                        yt = yout.tile([128, 512], F32, tag="y", name="yt")
                        # alternate PSUM evacuation between DVE and ACT so
                        # neither serializes the 8 groups of this block
                        if half == 0:
                            nc.vector.tensor_copy(yt[:], ps[:])
                        else:
                            nc.scalar.activation(yt[:], ps[:], Identity,
                                                 bias=0.0, scale=1.0)
                        nc.sync.dma_start(
                            y[t0:t0 + 128, half * 512:half * 512 + 512],
                            yt[:])

    nc.compile()
    return nc


def make_in_maps(x, Wq, bq, Wkv, bkv, Wo, bo):
    import ml_dtypes

    bf = ml_dtypes.bfloat16
    x = np.asarray(x, np.float32)
    Wq = np.asarray(Wq, np.float32)
    bq = np.asarray(bq, np.float32)
    Wkv = np.asarray(Wkv, np.float32)
    bkv = np.asarray(bkv, np.float32)
    Wo = np.asarray(Wo, np.float32)

    Wk, Wv = Wkv[:D], Wkv[D:]
    bk, bv = bkv[:D], bkv[D:]

    # causal triangle keep-mask: mask[p, u] = 1 iff u >= p
    u = np.arange(128)[None, :]
    kk = np.arange(128)[:, None]
    mask = (u >= kk).astype(bf)

    in_maps = []
    for c in range(NCORES):
        b = c // (NCORES // B)
        hs = HPC * (c % (NCORES // B))
        rows = slice(hs * HD, hs * HD + HPC * HD)
        bq_c = bq[rows] * SCALE
        # bqz/sclz: per-head column, live on that head's 64 partitions only
        bqz = np.zeros((128, HPC), np.float32)
        sclz = np.zeros((128, HPC), np.float32)
        for h in range(HPC):
            po = (h % 2) * 64
            m = h // 2
            bqz[po:po + 64, h] = bq_c[m * 128 + po:m * 128 + po + 64]
            sclz[po:po + 64, h] = SCALE
        in_maps.append({
            "xT": np.ascontiguousarray(x[b].T).astype(bf),
            "wqT": np.ascontiguousarray(Wq[rows].T).astype(bf),
            "wkT": np.ascontiguousarray(Wk[rows].T).astype(bf),
            "wvT": np.ascontiguousarray(Wv[rows].T).astype(bf),
            "woT": np.ascontiguousarray(Wo[:, rows].T).astype(bf),
            "bqz": bqz,
            "sclz": sclz,
            "bk": np.ascontiguousarray(bk[rows]),
            "mask": mask,
        })
    return in_maps


_NC_CACHE = None


def _get_nc():
    global _NC_CACHE
    if _NC_CACHE is None:
        _NC_CACHE = build_kernel()
    return _NC_CACHE


def kernel(x, Wq, bq, Wkv, bkv, Wo, bo, _trace=False, _trace_kwargs=None):
    nc = _get_nc()
    in_maps = make_in_maps(x, Wq, bq, Wkv, bkv, Wo, bo)
    kwargs = {}
    if _trace:
        kwargs = dict(trace=True, trace_cores=list(range(NCORES)),
                      **(_trace_kwargs or {}))
    res = run_bass_kernel_spmd(nc, in_maps, core_ids=list(range(NCORES)),
                               **kwargs)
    out = np.zeros((B, N, D), np.float32)
    for c, r in enumerate(res.results):
        out[c // (NCORES // B)] += r["y"]
    # bo plus the folded V-bias contribution (softmax rows sum to 1, so the
    # v-bias adds exactly bv @ Wo.T to every token)
    bv = np.asarray(bkv, np.float32)[D:]
    bo2 = np.asarray(bo, np.float32) + np.asarray(Wo, np.float32) @ bv
    out += bo2[None, None, :]
    if _trace:
        kernel.last_results = res
    return out


# revision 7
# speedup vs baseline: 1.0681x; 1.0681x over previous
"""Trainium2 Bass kernel for causal multi-head attention (b=2, n=2048, d=1024, h=16).

Sharding: 8 cores; core c handles batch (c // 4) and the 4 heads
[4*(c%4), 4*(c%4)+4).  Each core computes its heads' attention plus its
partial output projection y_part = O_heads @ Wo[:, cols].T ; the host sums
the per-batch partials and adds bo (with the V-bias contribution folded in
host-side: softmax rows sum to 1, so bv contributes exactly bv @ Wo.T).

All matmul operands are bf16 (fp32 PSUM accumulation); inputs are converted
host-side, halving the startup DMA vs f32r and enabling fast weight load.

Structure: the scalar-engine exp is the pacer of the attention phase, so the
projection matmuls for block b+1 and the output projection of block q-1 are
woven INTO block q's attention stream as PE filler units — the PE works
through fillers while ACT chews exp tiles.  Startup is DMA-chased: weight
and x chunks stream in k-chunk order and the first projection matmuls wait
on per-chunk DMA semaphores (no junk warm-up needed).

Per-core pipeline:
  xT (d-major, bf16) -> QTz, KT [hd, n] and V [n, hd] projections (bf16)
  ST tile [k,q] = KT-chunk.T x QTz_h       (K=128: both heads' KT rows with
                                            the other head's QT rows zeroed;
                                            1/8 scale folded into QT)
  PT = exp(ST) in bf16 (no max subtraction; scores are O(10), fp32-exp safe)
  causal masking: the diagonal 128-col triangle of each diagonal chunk is
  multiplied with a single [128,128] 0/1 bf16 mask block
  OT_aug [65, q] += V_aug-chunk.T x PT     (V_aug = [V | ones]; row 64 = l)
  normalize per head pair: l rows copied off PSUM, one batched reciprocal,
  one DRAM-bounce broadcast DMA, two PSUM-read muls into the OTn2 pair tile
  (heads 2p, 2p+1 stacked -> K=128, no zero rows)
  y[tok, :] += OTn2-pair-chunk.T x Wo_pair ; y is bf16 (host sums partials)
  last q-block: pair-0's output projection runs during pair-1's attention
  (separate y1 partial tensor) so the tail only waits on the final pair.
"""

import numpy as np

import concourse.bass as bass
import concourse.mybir as mybir
import concourse.tile as tile
from concourse import bacc
from concourse.bass_utils import run_bass_kernel_spmd

D = 1024          # d_model
N = 2048          # sequence length
B = 2             # batch
H_TOT = 16        # total heads
HD = 64           # head dim
HPC = 4           # heads per core
NCORES = 8
SCALE = HD ** -0.5

F32 = mybir.dt.float32
BF16 = mybir.dt.bfloat16

QTILE = 512       # q-tile width (free dim of score matmuls)
KCH = 128         # k-chunk (partition dim of score tiles)
NQT = N // QTILE  # 4
DCH = D // 128    # 8 d_model chunks
VROW = HD + 1     # V columns per head incl. ones column


def build_kernel():
    nc = bacc.Bacc("TRN2", target_bir_lowering=False, debug=False,
                   num_devices=NCORES)

    xT = nc.dram_tensor("xT", [D, N], BF16, kind="ExternalInput").ap()
    wq = nc.dram_tensor("wqT", [D, HPC * HD], BF16, kind="ExternalInput").ap()
    wk = nc.dram_tensor("wkT", [D, HPC * HD], BF16, kind="ExternalInput").ap()
    wv = nc.dram_tensor("wvT", [D, HPC * HD], BF16, kind="ExternalInput").ap()
    wo = nc.dram_tensor("woT", [HPC * HD, D], BF16, kind="ExternalInput").ap()
    bqz = nc.dram_tensor("bqz", [128, HPC], F32, kind="ExternalInput").ap()
    sclz = nc.dram_tensor("sclz", [128, HPC], F32, kind="ExternalInput").ap()
    bkd = nc.dram_tensor("bk", [HPC * HD], F32, kind="ExternalInput").ap()
    maskd = nc.dram_tensor("mask", [128, 128], BF16, kind="ExternalInput").ap()
    y = nc.dram_tensor("y", [N, D], BF16, kind="ExternalOutput").ap()
    # pair-1 partial of the last q-block (host adds it)
    y1 = nc.dram_tensor("y1", [QTILE, D], BF16, kind="ExternalOutput").ap()

    Exp = mybir.ActivationFunctionType.Exp
    Identity = mybir.ActivationFunctionType.Identity

    with tile.TileContext(nc) as tc:
        from collections import deque
        from contextlib import ExitStack
        with ExitStack() as ctx:
            singles = ctx.enter_context(tc.tile_pool(name="singles", bufs=1))
            pt_pool = ctx.enter_context(tc.tile_pool(name="pt", bufs=4))
            r_pool = ctx.enter_context(tc.tile_pool(name="rp", bufs=2))
            yout = ctx.enter_context(tc.tile_pool(name="yout", bufs=3))
            dram = ctx.enter_context(
                tc.tile_pool(name="dram", bufs=2, space="DRAM"))
            ps_mm = ctx.enter_context(
                tc.tile_pool(name="psmm", bufs=2, space="PSUM"))
            ps_st = ctx.enter_context(
                tc.tile_pool(name="psst", bufs=2, space="PSUM"))
            ps_ot = ctx.enter_context(
                tc.tile_pool(name="psot", bufs=2, space="PSUM"))

            # --- resident tiles -------------------------------------------
            bqz_sb = singles.tile([128, HPC], F32)
            sclz_sb = singles.tile([128, HPC], F32)
            bk_sb = singles.tile([128, 2], F32)
            mask_sb = singles.tile([128, 128], BF16)
            wq_sb = singles.tile([128, DCH, HPC * HD], BF16)
            wk_sb = singles.tile([128, DCH, HPC * HD], BF16)
            wv_sb = singles.tile([128, DCH, HPC * HD], BF16)
            xk = [[singles.tile([128, QTILE], BF16, name=f"xk{k}_{b}")
                   for b in range(NQT)] for k in range(DCH)]
            wo2 = [singles.tile([128, D], BF16, name=f"wo{p}")
                   for p in range(2)]
            QTz = [[singles.tile([128, QTILE], BF16, name=f"qtz{h}_{i}")
                    for i in range(NQT)] for h in range(HPC)]
            KT_sb = [singles.tile([128, 2, QTILE], BF16, name=f"kt{i}")
                     for i in range(NQT)]
            V_sb = [singles.tile([128, 4, HPC, VROW], BF16, name=f"v{i}")
                    for i in range(NQT)]
            # OTn2[p]: heads 2p (rows 0..63) and 2p+1 (rows 64..127) stacked
            OTn2 = [singles.tile([128, N], BF16, name=f"otn{p}")
                    for p in range(2)]

            # --- DMA stream (order = priority; compute chases it) ---------
            nc.sync.dma_start(bqz_sb[:], bqz)
            nc.sync.dma_start(sclz_sb[:], sclz)
            nc.sync.dma_start(bk_sb[:], bkd.rearrange("(o p) -> p o", p=128))
            for k in range(DCH):
                r = slice(k * 128, (k + 1) * 128)
                nc.sync.dma_start(wq_sb[:, k, :], wq[r, :])
                nc.sync.dma_start(wk_sb[:, k, :], wk[r, :])
                nc.sync.dma_start(xk[k][0][:], xT[r, 0:QTILE])
            for k in range(DCH):
                nc.sync.dma_start(wv_sb[:, k, :],
                                  wv[k * 128:(k + 1) * 128, :])
            for k in range(DCH):
                nc.sync.dma_start(xk[k][1][:],
                                  xT[k * 128:(k + 1) * 128, QTILE:2 * QTILE])
            for p in range(2):
                nc.sync.dma_start(wo2[p][:], wo[p * 128:(p + 1) * 128, :])
            nc.sync.dma_start(mask_sb[:], maskd)
            for b in range(2, NQT):
                for k in range(DCH):
                    nc.sync.dma_start(
                        xk[k][b][:],
                        xT[k * 128:(k + 1) * 128,
                           b * QTILE:(b + 1) * QTILE])

            # --- PE filler units ------------------------------------------
            fillers = deque()

            def pump(n=1):
                for _ in range(n):
                    if not fillers:
                        return
                    fillers.popleft()()

            def q_unit(blk, m):
                ps = ps_mm.tile([128, 512], F32, tag="mm", name="psq")
                for k in range(DCH):
                    nc.tensor.matmul(
                        ps[:], lhsT=wq_sb[:, k, m * 128:(m + 1) * 128],
                        rhs=xk[k][blk][:],
                        start=(k == 0), stop=(k == DCH - 1))
                for hh in range(2):
                    h = 2 * m + hh
                    # per-partition scale zeroes the other head's rows
                    nc.scalar.activation(
                        QTz[h][blk][:], ps[:], Identity,
                        bias=bqz_sb[:, h:h + 1], scale=sclz_sb[:, h:h + 1])

            def k_unit(blk, m):
                ps = ps_mm.tile([128, 512], F32, tag="mm", name="psk")
                for k in range(DCH):
                    nc.tensor.matmul(
                        ps[:], lhsT=wk_sb[:, k, m * 128:(m + 1) * 128],
                        rhs=xk[k][blk][:],
                        start=(k == 0), stop=(k == DCH - 1))
                nc.scalar.activation(
                    KT_sb[blk][:, m, :], ps[:], Identity,
                    bias=bk_sb[:, m:m + 1], scale=1.0)

            def v_unit(blk, tt):
                if tt == 0:
                    nc.vector.memset(V_sb[blk][:, :, :, HD], 1.0)
                ps = ps_mm.tile([128, 512], F32, tag="mm", name="psv")
                for k in range(DCH):
                    nc.tensor.matmul(
                        ps[:, :HPC * HD],
                        lhsT=xk[k][blk][:, tt * 128:(tt + 1) * 128],
                        rhs=wv_sb[:, k, :],
                        start=(k == 0), stop=(k == DCH - 1))
                nc.vector.tensor_copy(
                    V_sb[blk][:, tt, :, :HD], ps[:, :HPC * HD])

            def outproj_unit(qi, tt, half, pairs=(0, 1), ydst=None, yrow0=0):
                t0 = qi * QTILE + tt * 128
                ps = ps_mm.tile([128, 512], F32, tag="mm", name="psy")
                for i, p in enumerate(pairs):
                    nc.tensor.matmul(
                        ps[:], lhsT=OTn2[p][:, t0:t0 + 128],
                        rhs=wo2[p][:, half * 512:half * 512 + 512],
                        start=(i == 0), stop=(i == len(pairs) - 1))
                yt = yout.tile([128, 512], BF16, tag="y", name="yt")
                # alternate PSUM evacuation between DVE and ACT
                if (tt + half) % 2 == 0:
                    nc.vector.tensor_copy(yt[:], ps[:])
                else:
                    nc.scalar.activation(yt[:], ps[:], Identity,
                                         bias=0.0, scale=1.0)
                dst = y if ydst is None else ydst
                nc.sync.dma_start(
                    dst[t0 - yrow0:t0 - yrow0 + 128,
                        half * 512:half * 512 + 512], yt[:])

            # --- attention ------------------------------------------------
            def attention_head(qi, h):
                q0 = qi * QTILE
                mi = h // 2
                nprs = 2 * (qi + 1)        # pairs of 128-k-chunks
                pso = ps_ot.tile([VROW, 512], F32, tag="ot", name="pso")
                pair_ps = [None] * nprs
                pair_pt = [None] * nprs

                def consume(pi):
                    # last pair of each q-tile: only columns q >= r are
                    # unmasked (r = 256, 384); compute just those.
                    shrunk = (pi == nprs - 1)
                    pss, pt = pair_ps[pi], pair_pt[pi]
                    if shrunk:
                        for j in range(2):
                            r = (2 * pi + j) * KCH - q0
                            nc.scalar.activation(
                                pt[:, j, r:], pss[:, j, r:], Exp)
                    else:
                        nc.scalar.activation(pt[:], pss[:], Exp)
                    for j in range(2):
                        ki = 2 * pi + j
                        r = ki * KCH - q0
                        if r >= 0:
                            # causal triangle: the same [128,128] block
                            # masks the diagonal cols for every offset r
                            nc.vector.tensor_mul(
                                pt[:, j, r:r + KCH], pt[:, j, r:r + KCH],
                                mask_sb[:])
                    for j in range(2):
                        ki = 2 * pi + j
                        s = max(ki * KCH - q0, 0)
                        nc.tensor.matmul(
                            pso[:, s:],
                            lhsT=V_sb[ki // 4][:, ki % 4, h, :],
                            rhs=pt[:, j, s:],
                            start=(ki == 0), stop=(ki == 4 * (qi + 1) - 1))

                for pi in range(nprs):
                    pss = ps_st.tile([128, 2, 512], F32, tag="st",
                                     name="pss")
                    pair_ps[pi] = pss
                    pair_pt[pi] = pt_pool.tile([128, 2, 512], BF16,
                                               tag="pt", name="pt")
                    for j in range(2):
                        ki = 2 * pi + j
                        s = (ki * KCH - q0) if pi == nprs - 1 else 0
                        nc.tensor.matmul(
                            pss[:, j, s:],
                            lhsT=KT_sb[ki // 4][:, mi,
                                                (ki % 4) * 128:
                                                (ki % 4) * 128 + 128],
                            rhs=QTz[h][qi][:, s:],
                            start=True, stop=True)
                    if pi > 0:
                        consume(pi - 1)
                    pump(1)
                consume(nprs - 1)
                return pso

            def normalize_head(qi, h, pso):
                q0 = qi * QTILE
                p, po = h // 2, (h % 2) * HD
                lw = r_pool.tile([1, 512], F32, tag="lw", name="lw1")
                nc.vector.tensor_copy(lw[:], pso[HD:HD + 1, :])
                rl = r_pool.tile([1, 512], F32, tag="rl", name="rl1")
                nc.vector.reciprocal_approx_fast(out=rl[:], in_=lw[:])
                sc = dram.tile([1, 512], F32, tag="sc", name="sc1")
                nc.sync.dma_start(sc[:], rl[:])
                rb = r_pool.tile([HD, 512], F32, tag="rb", name="rb1")
                row = sc[0, :]
                bcast = bass.AP(tensor=row.tensor, offset=row.offset,
                                ap=[[0, HD]] + list(row.ap))
                nc.sync.dma_start(rb[:], bcast)
                nc.vector.tensor_mul(
                    OTn2[p][po:po + HD, q0:q0 + QTILE], pso[:HD, :], rb[:])

            # --- schedule -------------------------------------------------
            # block-0 projections run directly (nothing else to do yet)
            for m in range(2):
                q_unit(0, m)
                k_unit(0, m)
            for tt in range(4):
                v_unit(0, tt)

            for qi in range(NQT):
                # queue fillers: next block's projections, then the
                # previous block's output projection (its normalize has
                # just been emitted, so give it a little head start)
                if qi + 1 < NQT:
                    for m in range(2):
                        fillers.append(
                            lambda b=qi + 1, m=m: q_unit(b, m))
                        fillers.append(
                            lambda b=qi + 1, m=m: k_unit(b, m))
                if qi >= 1:
                    for tt in range(4):
                        for half in range(2):
                            fillers.append(
                                lambda q=qi - 1, t=tt, hf=half:
                                outproj_unit(q, t, hf))
                if qi + 1 < NQT:
                    for tt in range(4):
                        fillers.append(
                            lambda b=qi + 1, t=tt: v_unit(b, t))

                last = (qi == NQT - 1)
                for hp in range(2):
                    pso_a = attention_head(qi, 2 * hp)
                    normalize_head(qi, 2 * hp, pso_a)
                    pso_b = attention_head(qi, 2 * hp + 1)
                    normalize_head(qi, 2 * hp + 1, pso_b)
                    if last and hp == 0:
                        # pair-0 output projection of the last block runs
                        # during pair-1's attention, into its own partial
                        for tt in range(4):
                            for half in range(2):
                                fillers.append(
                                    lambda t=tt, hf=half:
                                    outproj_unit(NQT - 1, t, hf,
                                                 pairs=(0,)))
                # all remaining fillers must land before the next block's
                # attention (it needs the projections)
                pump(len(fillers))

            # tail: pair-1 output projection of the last block
            for tt in range(4):
                for half in range(2):
                    outproj_unit(NQT - 1, tt, half, pairs=(1,), ydst=y1,
                                 yrow0=(NQT - 1) * QTILE)

    nc.compile()
    return nc


def make_in_maps(x, Wq, bq, Wkv, bkv, Wo, bo):
    import ml_dtypes

    bf = ml_dtypes.bfloat16
    x = np.asarray(x, np.float32)
    Wq = np.asarray(Wq, np.float32)
    bq = np.asarray(bq, np.float32)
    Wkv = np.asarray(Wkv, np.float32)
    bkv = np.asarray(bkv, np.float32)
    Wo = np.asarray(Wo, np.float32)

    Wk, Wv = Wkv[:D], Wkv[D:]
    bk, bv = bkv[:D], bkv[D:]

    # causal triangle keep-mask: mask[p, u] = 1 iff u >= p
    u = np.arange(128)[None, :]
    kk = np.arange(128)[:, None]
    mask = (u >= kk).astype(bf)

    in_maps = []
    for c in range(NCORES):
        b = c // (NCORES // B)
        hs = HPC * (c % (NCORES // B))
        rows = slice(hs * HD, hs * HD + HPC * HD)
        bq_c = bq[rows] * SCALE
        # bqz/sclz: per-head column, live on that head's 64 partitions only
        bqzv = np.zeros((128, HPC), np.float32)
        sclzv = np.zeros((128, HPC), np.float32)
        for h in range(HPC):
            po = (h % 2) * 64
            m = h // 2
            bqzv[po:po + 64, h] = bq_c[m * 128 + po:m * 128 + po + 64]
            sclzv[po:po + 64, h] = SCALE
        in_maps.append({
            "xT": np.ascontiguousarray(x[b].T).astype(bf),
            "wqT": np.ascontiguousarray(Wq[rows].T).astype(bf),
            "wkT": np.ascontiguousarray(Wk[rows].T).astype(bf),
            "wvT": np.ascontiguousarray(Wv[rows].T).astype(bf),
            "woT": np.ascontiguousarray(Wo[:, rows].T).astype(bf),
            "bqz": bqzv,
            "sclz": sclzv,
            "bk": np.ascontiguousarray(bk[rows]),
            "mask": mask,
        })
    return in_maps


_NC_CACHE = None


def _get_nc():
    global _NC_CACHE
    if _NC_CACHE is None:
        _NC_CACHE = build_kernel()
    return _NC_CACHE


def kernel(x, Wq, bq, Wkv, bkv, Wo, bo, _trace=False, _trace_kwargs=None):
    nc = _get_nc()
    in_maps = make_in_maps(x, Wq, bq, Wkv, bkv, Wo, bo)
    kwargs = {}
    if _trace:
        kwargs = dict(trace=True, trace_cores=list(range(NCORES)),
                      **(_trace_kwargs or {}))
    res = run_bass_kernel_spmd(nc, in_maps, core_ids=list(range(NCORES)),
                               **kwargs)
    out = np.zeros((B, N, D), np.float32)
    for c, r in enumerate(res.results):
        b = c // (NCORES // B)
        out[b] += np.asarray(r["y"], np.float32)
        out[b, (NQT - 1) * QTILE:] += np.asarray(r["y1"], np.float32)
    # bo plus the folded V-bias contribution (softmax rows sum to 1, so the
    # v-bias adds exactly bv @ Wo.T to every token)
    bv = np.asarray(bkv, np.float32)[D:]
    bo2 = np.asarray(bo, np.float32) + np.asarray(Wo, np.float32) @ bv
    out += bo2[None, None, :]
    if _trace:
        kernel.last_results = res
    return out


# revision 9
# speedup vs baseline: 1.2079x; 1.1309x over previous
"""Trainium2 Bass kernel for causal multi-head attention (b=2, n=2048, d=1024, h=16).

Sharding: 8 cores; core c handles batch (c // 4) and the 4 heads
[4*(c%4), 4*(c%4)+4).  Each core computes its heads' attention plus its
partial output projection y_part = O_heads @ Wo[:, cols].T ; the host sums
the per-batch partials and adds bo (with the V-bias contribution folded in
host-side: softmax rows sum to 1, so bv contributes exactly bv @ Wo.T).

All matmul operands are bf16 (fp32 PSUM accumulation); inputs are converted
host-side, halving the startup DMA vs f32r and enabling fast weight load.

Structure: the scalar-engine exp is the pacer of the attention phase, so the
projection matmuls for block b+1 and the output projection of block q-1 are
woven INTO block q's attention stream as PE filler units — the PE works
through fillers while ACT chews exp tiles.  Startup is DMA-chased: the
merged wqkv weight and x chunks stream in k-chunk order and the first
projection matmuls wait on per-chunk DMA semaphores (no junk warm-up).

Per-core pipeline:
  xT (d-major, bf16) -> QTz, KT [hd, n] and V [n, hd] projections (bf16)
  ST tile [k,q] = KT-chunk.T x QTz_h       (K=128: both heads' KT rows with
                                            the other head's QT rows zeroed;
                                            1/8 scale folded into QT)
  PT = exp(ST) in bf16 (no max subtraction; scores are O(10), fp32-exp safe)
  causal masking: the diagonal 128-col triangle of each diagonal chunk is
  multiplied with a single [128,128] 0/1 bf16 mask block
  OT_aug [65, q] += V_aug-chunk.T x PT     (V_aug = [V | ones]; row 64 = l)
  normalize per head: l copied off PSUM (DVE), reciprocal (DVE), broadcast
  across 64 partitions on the otherwise-idle GpSimd engine
  (partition_broadcast), one fused PSUM-read mul into the OTn2 pair tile
  (heads 2p, 2p+1 stacked -> out-proj K=128 with no zero rows)
  y[tok, :] += OTn2-pair-chunk.T x Wo_pair ; y is bf16 (host sums partials)
  last q-block: pair-0's output projection runs during pair-1's attention
  (separate y1 partial tensor) so the tail only waits on the final pair.
"""

import numpy as np

import concourse.bass as bass
import concourse.mybir as mybir
import concourse.tile as tile
from concourse import bacc
from concourse.bass_utils import run_bass_kernel_spmd

D = 1024          # d_model
N = 2048          # sequence length
B = 2             # batch
H_TOT = 16        # total heads
HD = 64           # head dim
HPC = 4           # heads per core
NCORES = 8
SCALE = HD ** -0.5

F32 = mybir.dt.float32
BF16 = mybir.dt.bfloat16

QTILE = 512       # q-tile width (free dim of score matmuls)
KCH = 128         # k-chunk (partition dim of score tiles)
NQT = N // QTILE  # 4
DCH = D // 128    # 8 d_model chunks
VROW = HD + 1     # V columns per head incl. ones column
WQ0, WK0, WV0 = 0, HPC * HD, 2 * HPC * HD   # col offsets in merged wqkv


def build_kernel():
    nc = bacc.Bacc("TRN2", target_bir_lowering=False, debug=False,
                   num_devices=NCORES)

    xT = nc.dram_tensor("xT", [D, N], BF16, kind="ExternalInput").ap()
    wqkv = nc.dram_tensor("wqkvT", [D, 3 * HPC * HD], BF16,
                          kind="ExternalInput").ap()
    wo = nc.dram_tensor("woT", [HPC * HD, D], BF16, kind="ExternalInput").ap()
    bqz = nc.dram_tensor("bqz", [128, HPC], F32, kind="ExternalInput").ap()
    sclz = nc.dram_tensor("sclz", [128, HPC], F32, kind="ExternalInput").ap()
    bkd = nc.dram_tensor("bk", [HPC * HD], F32, kind="ExternalInput").ap()
    maskd = nc.dram_tensor("mask", [128, 128], BF16, kind="ExternalInput").ap()
    y = nc.dram_tensor("y", [N, D], BF16, kind="ExternalOutput").ap()
    # pair-1 partial of the last q-block (host adds it)
    y1 = nc.dram_tensor("y1", [QTILE, D], BF16, kind="ExternalOutput").ap()

    Exp = mybir.ActivationFunctionType.Exp
    Identity = mybir.ActivationFunctionType.Identity

    with tile.TileContext(nc) as tc:
        from collections import deque
        from contextlib import ExitStack
        with ExitStack() as ctx:
            singles = ctx.enter_context(tc.tile_pool(name="singles", bufs=1))
            pt_pool = ctx.enter_context(tc.tile_pool(name="pt", bufs=4))
            r_pool = ctx.enter_context(tc.tile_pool(name="rp", bufs=2))
            yout = ctx.enter_context(tc.tile_pool(name="yout", bufs=3))
            ps_mm = ctx.enter_context(
                tc.tile_pool(name="psmm", bufs=2, space="PSUM"))
            ps_st = ctx.enter_context(
                tc.tile_pool(name="psst", bufs=2, space="PSUM"))
            ps_ot = ctx.enter_context(
                tc.tile_pool(name="psot", bufs=2, space="PSUM"))

            # --- resident tiles -------------------------------------------
            bqz_sb = singles.tile([128, HPC], F32)
            sclz_sb = singles.tile([128, HPC], F32)
            bk_sb = singles.tile([128, 2], F32)
            mask_sb = singles.tile([128, 128], BF16)
            wqkv_sb = singles.tile([128, DCH, 3 * HPC * HD], BF16)
            xk = [[singles.tile([128, QTILE], BF16, name=f"xk{k}_{b}")
                   for b in range(NQT)] for k in range(DCH)]
            wo2 = [singles.tile([128, D], BF16, name=f"wo{p}")
                   for p in range(2)]
            QTz = [[singles.tile([128, QTILE], BF16, name=f"qtz{h}_{i}")
                    for i in range(NQT)] for h in range(HPC)]
            KT_sb = [singles.tile([128, 2, QTILE], BF16, name=f"kt{i}")
                     for i in range(NQT)]
            V_sb = [singles.tile([128, 4, HPC, VROW], BF16, name=f"v{i}")
                    for i in range(NQT)]
            # OTn2[p]: heads 2p (rows 0..63) and 2p+1 (rows 64..127) stacked
            OTn2 = [singles.tile([128, N], BF16, name=f"otn{p}")
                    for p in range(2)]

            # --- DMA stream (order = priority; compute chases it) ---------
            nc.sync.dma_start(bqz_sb[:], bqz)
            nc.sync.dma_start(sclz_sb[:], sclz)
            nc.sync.dma_start(bk_sb[:], bkd.rearrange("(o p) -> p o", p=128))
            for k in range(DCH):
                r = slice(k * 128, (k + 1) * 128)
                nc.sync.dma_start(wqkv_sb[:, k, :], wqkv[r, :])
                nc.sync.dma_start(xk[k][0][:], xT[r, 0:QTILE])
            nc.sync.dma_start(mask_sb[:], maskd)
            for k in range(DCH):
                nc.sync.dma_start(xk[k][1][:],
                                  xT[k * 128:(k + 1) * 128, QTILE:2 * QTILE])
            for p in range(2):
                nc.sync.dma_start(wo2[p][:], wo[p * 128:(p + 1) * 128, :])
            for b in range(2, NQT):
                for k in range(DCH):
                    nc.sync.dma_start(
                        xk[k][b][:],
                        xT[k * 128:(k + 1) * 128,
                           b * QTILE:(b + 1) * QTILE])

            # --- PE filler units ------------------------------------------
            fillers = deque()

            def pump(n=1):
                for _ in range(n):
                    if not fillers:
                        return
                    fillers.popleft()()

            def q_unit(blk, m):
                ps = ps_mm.tile([128, 512], F32, tag="mm", name="psq")
                for k in range(DCH):
                    nc.tensor.matmul(
                        ps[:],
                        lhsT=wqkv_sb[:, k, WQ0 + m * 128:WQ0 + (m + 1) * 128],
                        rhs=xk[k][blk][:],
                        start=(k == 0), stop=(k == DCH - 1))
                for hh in range(2):
                    h = 2 * m + hh
                    # per-partition scale zeroes the other head's rows
                    nc.scalar.activation(
                        QTz[h][blk][:], ps[:], Identity,
                        bias=bqz_sb[:, h:h + 1], scale=sclz_sb[:, h:h + 1])

            def k_unit(blk, m):
                ps = ps_mm.tile([128, 512], F32, tag="mm", name="psk")
                for k in range(DCH):
                    nc.tensor.matmul(
                        ps[:],
                        lhsT=wqkv_sb[:, k, WK0 + m * 128:WK0 + (m + 1) * 128],
                        rhs=xk[k][blk][:],
                        start=(k == 0), stop=(k == DCH - 1))
                nc.scalar.activation(
                    KT_sb[blk][:, m, :], ps[:], Identity,
                    bias=bk_sb[:, m:m + 1], scale=1.0)

            def v_unit(blk, tt):
                if tt == 0:
                    nc.vector.memset(V_sb[blk][:, :, :, HD], 1.0)
                ps = ps_mm.tile([128, 512], F32, tag="mm", name="psv")
                for k in range(DCH):
                    nc.tensor.matmul(
                        ps[:, :HPC * HD],
                        lhsT=xk[k][blk][:, tt * 128:(tt + 1) * 128],
                        rhs=wqkv_sb[:, k, WV0:WV0 + HPC * HD],
                        start=(k == 0), stop=(k == DCH - 1))
                nc.vector.tensor_copy(
                    V_sb[blk][:, tt, :, :HD], ps[:, :HPC * HD])

            def outproj_unit(qi, tt, pairs=(0, 1), ydst=None, yrow0=0):
                t0 = qi * QTILE + tt * 128
                yt = yout.tile([128, 2, 512], BF16, tag="y", name="yt")
                for half in range(2):
                    ps = ps_mm.tile([128, 512], F32, tag="mm", name="psy")
                    for i, p in enumerate(pairs):
                        nc.tensor.matmul(
                            ps[:], lhsT=OTn2[p][:, t0:t0 + 128],
                            rhs=wo2[p][:, half * 512:half * 512 + 512],
                            start=(i == 0), stop=(i == len(pairs) - 1))
                    # alternate PSUM evacuation between DVE and ACT
                    if half == 0:
                        nc.vector.tensor_copy(yt[:, half, :], ps[:])
                    else:
                        nc.scalar.activation(yt[:, half, :], ps[:], Identity,
                                             bias=0.0, scale=1.0)
                dst = y if ydst is None else ydst
                nc.sync.dma_start(dst[t0 - yrow0:t0 - yrow0 + 128, :], yt[:])

            # --- attention ------------------------------------------------
            def attention_head(qi, h):
                q0 = qi * QTILE
                mi = h // 2
                nprs = 2 * (qi + 1)        # pairs of 128-k-chunks
                pso = ps_ot.tile([VROW, 512], F32, tag="ot", name="pso")
                pair_ps = [None] * nprs
                pair_pt = [None] * nprs

                def consume(pi):
                    # last pair of each q-tile: only columns q >= r are
                    # unmasked (r = 256, 384); compute just those.
                    shrunk = (pi == nprs - 1)
                    pss, pt = pair_ps[pi], pair_pt[pi]
                    if shrunk:
                        for j in range(2):
                            r = (2 * pi + j) * KCH - q0
                            nc.scalar.activation(
                                pt[:, j, r:], pss[:, j, r:], Exp)
                    else:
                        nc.scalar.activation(pt[:], pss[:], Exp)
                    for j in range(2):
                        ki = 2 * pi + j
                        r = ki * KCH - q0
                        if r >= 0:
                            # causal triangle: the same [128,128] block
                            # masks the diagonal cols for every offset r
                            nc.vector.tensor_mul(
                                pt[:, j, r:r + KCH], pt[:, j, r:r + KCH],
                                mask_sb[:])
                    for j in range(2):
                        ki = 2 * pi + j
                        s = max(ki * KCH - q0, 0)
                        nc.tensor.matmul(
                            pso[:, s:],
                            lhsT=V_sb[ki // 4][:, ki % 4, h, :],
                            rhs=pt[:, j, s:],
                            start=(ki == 0), stop=(ki == 4 * (qi + 1) - 1))

                for pi in range(nprs):
                    pss = ps_st.tile([128, 2, 512], F32, tag="st",
                                     name="pss")
                    pair_ps[pi] = pss
                    pair_pt[pi] = pt_pool.tile([128, 2, 512], BF16,
                                               tag="pt", name="pt")
                    for j in range(2):
                        ki = 2 * pi + j
                        s = (ki * KCH - q0) if pi == nprs - 1 else 0
                        nc.tensor.matmul(
                            pss[:, j, s:],
                            lhsT=KT_sb[ki // 4][:, mi,
                                                (ki % 4) * 128:
                                                (ki % 4) * 128 + 128],
                            rhs=QTz[h][qi][:, s:],
                            start=True, stop=True)
                    if pi > 0:
                        consume(pi - 1)
                    pump(1)
                consume(nprs - 1)
                return pso

            def normalize_head(qi, h, pso):
                q0 = qi * QTILE
                p, po = h // 2, (h % 2) * HD
                lw = r_pool.tile([1, 512], F32, tag="lw", name="lw1")
                nc.vector.tensor_copy(lw[:], pso[HD:HD + 1, :])
                rl = r_pool.tile([1, 512], F32, tag="rl", name="rl1")
                nc.vector.reciprocal_approx_fast(out=rl[:], in_=lw[:])
                rb = r_pool.tile([HD, 512], F32, tag="rb", name="rb1")
                nc.gpsimd.partition_broadcast(rb[:], rl[:])
                nc.vector.tensor_mul(
                    OTn2[p][po:po + HD, q0:q0 + QTILE], pso[:HD, :], rb[:])

            # --- schedule -------------------------------------------------
            # block-0 projections run directly (nothing else to do yet)
            for m in range(2):
                q_unit(0, m)
                k_unit(0, m)
            for tt in range(4):
                v_unit(0, tt)

            for qi in range(NQT):
                # queue fillers: next block's projections, then the
                # previous block's output projection
                if qi + 1 < NQT:
                    for m in range(2):
                        fillers.append(
                            lambda b=qi + 1, m=m: q_unit(b, m))
                        fillers.append(
                            lambda b=qi + 1, m=m: k_unit(b, m))
                if qi >= 1:
                    for tt in range(4):
                        fillers.append(
                            lambda q=qi - 1, t=tt: outproj_unit(q, t))
                if qi + 1 < NQT:
                    for tt in range(4):
                        fillers.append(
                            lambda b=qi + 1, t=tt: v_unit(b, t))

                last = (qi == NQT - 1)
                for hp in range(2):
                    pso_a = attention_head(qi, 2 * hp)
                    normalize_head(qi, 2 * hp, pso_a)
                    pso_b = attention_head(qi, 2 * hp + 1)
                    normalize_head(qi, 2 * hp + 1, pso_b)
                    if last and hp == 0:
                        # pair-0 output projection of the last block runs
                        # during pair-1's attention, into its own partial
                        for tt in range(4):
                            fillers.append(
                                lambda t=tt:
                                outproj_unit(NQT - 1, t, pairs=(0,)))
                # all remaining fillers must land before the next block's
                # attention (it needs the projections)
                pump(len(fillers))

            # tail: pair-1 output projection of the last block
            for tt in range(4):
                outproj_unit(NQT - 1, tt, pairs=(1,), ydst=y1,
                             yrow0=(NQT - 1) * QTILE)

    nc.compile()
    return nc


def make_in_maps(x, Wq, bq, Wkv, bkv, Wo, bo):
    import ml_dtypes

    bf = ml_dtypes.bfloat16
    x = np.asarray(x, np.float32)
    Wq = np.asarray(Wq, np.float32)
    bq = np.asarray(bq, np.float32)
    Wkv = np.asarray(Wkv, np.float32)
    bkv = np.asarray(bkv, np.float32)
    Wo = np.asarray(Wo, np.float32)

    Wk, Wv = Wkv[:D], Wkv[D:]
    bk, bv = bkv[:D], bkv[D:]

    # causal triangle keep-mask: mask[p, u] = 1 iff u >= p
    u = np.arange(128)[None, :]
    kk = np.arange(128)[:, None]
    mask = (u >= kk).astype(bf)

    in_maps = []
    for c in range(NCORES):
        b = c // (NCORES // B)
        hs = HPC * (c % (NCORES // B))
        rows = slice(hs * HD, hs * HD + HPC * HD)
        bq_c = bq[rows] * SCALE
        # bqz/sclz: per-head column, live on that head's 64 partitions only
        bqzv = np.zeros((128, HPC), np.float32)
        sclzv = np.zeros((128, HPC), np.float32)
        for h in range(HPC):
            po = (h % 2) * 64
            m = h // 2
            bqzv[po:po + 64, h] = bq_c[m * 128 + po:m * 128 + po + 64]
            sclzv[po:po + 64, h] = SCALE
        wqkv_c = np.concatenate(
            [Wq[rows].T, Wk[rows].T, Wv[rows].T], axis=1)
        in_maps.append({
            "xT": np.ascontiguousarray(x[b].T).astype(bf),
            "wqkvT": np.ascontiguousarray(wqkv_c).astype(bf),
            "woT": np.ascontiguousarray(Wo[:, rows].T).astype(bf),
            "bqz": bqzv,
            "sclz": sclzv,
            "bk": np.ascontiguousarray(bk[rows]),
            "mask": mask,
        })
    return in_maps


_NC_CACHE = None


def _get_nc():
    global _NC_CACHE
    if _NC_CACHE is None:
        _NC_CACHE = build_kernel()
    return _NC_CACHE


def kernel(x, Wq, bq, Wkv, bkv, Wo, bo, _trace=False, _trace_kwargs=None):
    nc = _get_nc()
    in_maps = make_in_maps(x, Wq, bq, Wkv, bkv, Wo, bo)
    kwargs = {}
    if _trace:
        kwargs = dict(trace=True, trace_cores=list(range(NCORES)),
                      **(_trace_kwargs or {}))
    res = run_bass_kernel_spmd(nc, in_maps, core_ids=list(range(NCORES)),
                               **kwargs)
    out = np.zeros((B, N, D), np.float32)
    for c, r in enumerate(res.results):
        b = c // (NCORES // B)
        out[b] += np.asarray(r["y"], np.float32)
        out[b, (NQT - 1) * QTILE:] += np.asarray(r["y1"], np.float32)
    # bo plus the folded V-bias contribution (softmax rows sum to 1, so the
    # v-bias adds exactly bv @ Wo.T to every token)
    bv = np.asarray(bkv, np.float32)[D:]
    bo2 = np.asarray(bo, np.float32) + np.asarray(Wo, np.float32) @ bv
    out += bo2[None, None, :]
    if _trace:
        kernel.last_results = res
    return out
